# revision 40
# baseline (speedup 1.0000x reference)
"""Fused transformer block (nn_Block_2388001816768) on 8 Trainium2 NeuronCores.

Sharding: (batch, head-half) -> one core. Core c handles batch c//2 and
heads [8o, 8o+8) where o = c%2, over the FULL sequence. Causal attention
is exact (no masked-tile waste): q-chunk qc attends kv tiles 0..4qc+3
with a tril constant on the diagonal tile.

After the Wo projection each core holds a partial attention output
(its 8 heads' contribution) plus 0.5*(x + bo); a pairwise ReduceScatter
(add) between the two cores of a batch yields x2 = x + attn_out, split
so each core keeps its sequence half for LN2 + FFN.

Large matmuls run in float32r (TF32-like, full PE rate at free>=256),
fp32 accum. The FFN runs fp8e4 DoubleRow (2x PE rate): W1*16 / W2*64
are pre-scaled into fp8's normal range host-side; the inverse scales
fold into the GELU input scale and the output epilogue. LN scale/shift
and the 1/sqrt(HD) score scale are folded into projection weights
host-side.
"""

import numpy as np

import concourse.bacc as bacc
import concourse.bass as bass  # noqa: F401
import concourse.mybir as mybir
import concourse.tile as tile
from concourse.bass_utils import run_bass_kernel_spmd
from concourse.masks import make_identity

B, T, D, H = 4, 2048, 1024, 16
HD = D // H  # 64
FF = 4 * D  # 4096
TQ = T // 2  # output rows per core = 1024
P = 128
HL = H // 2  # heads per core = 8
HPL = HL // 2  # head pairs per core = 4

f32 = mybir.dt.float32
f32r = mybir.dt.float32r
bf16 = mybir.dt.bfloat16
fp8 = mybir.dt.float8e4
AF = mybir.ActivationFunctionType
ALU = mybir.AluOpType
DR = mybir.MatmulPerfMode.DoubleRow
W1S = 16.0  # host-side weight scale (fp8 range), undone by activation scale
W2S = 64.0

_CACHE = {}


def _emit_body(nc, tc, sfx, cst, x2d, p2d, dram):
    (xl_d, wq_d, wk_d, wv_d, wo_d, w1_d, w2_d, bo_d, b1f_d, b2_d, out_d) = dram
    ident, ident_b, tril, ones16, qkvb, eps = cst

    DT = D // P  # 8 d-tiles
    NT = T // P  # 16 t-tiles
    NQ = TQ // P  # 8 own-half q-tiles
    FT = FF // P  # 32 ff-tiles

    with tc.tile_pool(name="ctxp" + sfx, bufs=1) as ctxp:
        ctxT = ctxp.tile([P, HPL, T], f32r)  # ctx^T head-pair-stacked

        with tc.tile_pool(name="hTp" + sfx, bufs=1) as hTp:
            # h^T in 4 t-chunks of 512 so phase 2 can overlap phase 1
            hTc = []
            for i in range(4):
                hT_i = hTp.tile([P, DT, 512], f32r, tag=f"hT{i}")
                hTc.append(hT_i)

            # ---------- Phase 1: LN1 + transpose (full T) ----------
            with (
                tc.tile_pool(name="ln1" + sfx, bufs=3) as ln1,
                tc.tile_pool(name="ps1" + sfx, bufs=4, space="PSUM") as ps1,
            ):
                for tt in range(NT):
                    x_t = ln1.tile([P, D], f32, tag="x_t")
                    nc.sync.dma_start(x_t, xl_d[tt * P:(tt + 1) * P, :])
                    st = ln1.tile([P, 2, 6], f32, tag="st")
                    nc.vector.bn_stats(st[:, 0, :], x_t[:, 0:512])
                    nc.vector.bn_stats(st[:, 1, :], x_t[:, 512:1024])
                    mv = ln1.tile([P, 2], f32, tag="mv")
                    nc.vector.bn_aggr(mv, st)
                    rstd = ln1.tile([P, 1], f32, tag="rstd")
                    nc.scalar.activation(rstd, mv[:, 1:2], AF.Sqrt, bias=eps)
                    nc.vector.reciprocal(rstd, rstd)
                    nb = ln1.tile([P, 2], f32, tag="nb")
                    nc.vector.tensor_scalar_mul(nb[:, 0:1], rstd, -1.0)
                    nc.vector.tensor_mul(
                        nb[:, 1:2], mv[:, 0:1], nb[:, 0:1])
                    h_t = ln1.tile([P, D], f32r, tag="h_t")
                    tpos = tt % 4
                    for dh in range(2):
                        hsl = slice(dh * 512, (dh + 1) * 512)
                        nc.scalar.activation(
                            h_t[:, hsl], x_t[:, hsl], AF.Identity,
                            bias=nb[:, 1:2], scale=rstd)
                        tp = ps1.tile([P, 4, P], f32r, tag="tp")
                        for k in range(4):
                            dt = dh * 4 + k
                            nc.tensor.transpose(
                                tp[:, k, :],
                                h_t[:, dt * P:(dt + 1) * P], ident)
                        dst = hTc[tt // 4][:, dh * 4:dh * 4 + 4,
                                           tpos * P:(tpos + 1) * P]
                        if dh == 0:
                            nc.scalar.copy(dst, tp)
                        else:
                            nc.vector.tensor_copy(dst, tp)

            # ---------- Phase 2a: QKV projections for all head pairs ------
            # Even head of a pair in partitions 0:64, odd in 64:128 of the
            # shared tiles.
            qTs, kTs, vaugs = [], [], []
            with (
                tc.tile_pool(name="whead" + sfx, bufs=1) as whead,
                tc.tile_pool(name="vcp" + sfx, bufs=2) as vcp,
                tc.tile_pool(name="psqkv" + sfx, bufs=2, space="PSUM") as psqkv,
            ):
                for hp in range(HPL):
                    wp = whead.tile([P, 3, DT, 2 * HD], f32r, tag="wp")
                    for wi, w_dram in enumerate((wq_d, wk_d, wv_d)):
                        nc.sync.dma_start(
                            wp[:, wi],
                            w_dram[:, hp * 2 * HD:(hp + 1) * 2 * HD]
                            .rearrange("(dt q) m -> q dt m", q=P))

                    qT = ctxp.tile([P, T], f32r, tag=f"qT{hp}")
                    kT = ctxp.tile([P, T], f32r, tag=f"kT{hp}")
                    vaug_e = ctxp.tile([P, NT, HD + 1], bf16, tag=f"va{hp}e")
                    vaug_o = ctxp.tile([P, NT, HD + 1], bf16, tag=f"va{hp}o")
                    qTs.append(qT)
                    kTs.append(kT)
                    vaugs.append((vaug_e, vaug_o))
                    nc.vector.tensor_copy(
                        vaug_e[:, :, HD:HD + 1], ones16.unsqueeze(2))
                    nc.vector.tensor_copy(
                        vaug_o[:, :, HD:HD + 1], ones16.unsqueeze(2))
                    for (wi, bcol) in ((0, 3 * hp), (1, 3 * hp + 1),
                                       (2, 3 * hp + 2)):
                        for c in range(T // 512):
                            pp = psqkv.tile([P, 512], f32, tag="pp")
                            for dt in range(DT):
                                nc.tensor.matmul(
                                    pp, wp[:, wi, dt, :],
                                    hTc[c][:, dt, :],
                                    start=(dt == 0), stop=(dt == DT - 1))
                            csl = slice(c * 512, (c + 1) * 512)
                            if wi == 0:
                                nc.vector.tensor_scalar_add(
                                    out=qT[:, csl], in0=pp,
                                    scalar1=qkvb[:, bcol:bcol + 1])
                            elif wi == 1:
                                nc.vector.tensor_scalar_add(
                                    out=kT[:, csl], in0=pp,
                                    scalar1=qkvb[:, bcol:bcol + 1])
                            else:
                                vc = vcp.tile([P, 512], bf16, tag="vc")
                                nc.vector.tensor_scalar_add(
                                    out=vc, in0=pp,
                                    scalar1=qkvb[:, bcol:bcol + 1])
                                for k in range(4):
                                    kt = 4 * c + k
                                    vp = psqkv.tile([P, P], bf16, tag="vp")
                                    nc.tensor.transpose(
                                        vp, vc[:, k * P:(k + 1) * P],
                                        ident_b)
                                    nc.vector.tensor_copy(
                                        vaug_e[:, kt, 0:HD], vp[:, 0:HD])
                                    nc.vector.tensor_copy(
                                        vaug_o[:, kt, 0:HD], vp[:, HD:P])

        # ---------- Phase 2b: exact causal attn + Wo + ReduceScatter ------
        # q-chunk order (1,3,0,2): after qc1+qc3 the RS chunks 2,3 (row
        # tiles {4,5,12,13} / {6,7,14,15}) launch and hide under the
        # remaining attention; chunks 0,1 launch last and hide under the
        # LN2/FFN pass for half 1, which is consumed first.
        with (
            tc.tile_pool(name="pt" + sfx, bufs=4) as ptp,
            tc.tile_pool(name="wop" + sfx, bufs=1) as wop,
            tc.tile_pool(name="ph3" + sfx, bufs=3) as ph3,
            tc.tile_pool(name="pssc" + sfx, bufs=5, space="PSUM") as pssc,
            tc.tile_pool(name="psctx" + sfx, bufs=1, space="PSUM") as psctx,
            tc.tile_pool(name="ps3" + sfx, bufs=1, space="PSUM") as ps3,
        ):
            wo_sb = wop.tile([P, HPL, D], f32r)
            nc.sync.dma_start(
                wo_sb, wo_d.ap().rearrange("(pc p) n -> p pc n", p=P))
            bo_bc = wop.tile([P, D], f32)  # pre-halved host-side
            nc.sync.dma_start(bo_bc, bo_d.ap().to_broadcast([P, D]))
            for qc in (1, 3, 0, 2):
                qsl = slice(qc * 512, (qc + 1) * 512)
                kv_tiles = list(range(0, (qc + 1) * 4))
                for hp in range(HPL):
                    qT, kT = qTs[hp], kTs[hp]
                    vaug_e, vaug_o = vaugs[hp]
                    ctx_e = psctx.tile([HD + 1, 512], f32, tag="ctx_e")
                    ctx_o = psctx.tile([HD + 1, 512], f32, tag="ctx_o")
                    for n, i in enumerate(kv_tiles):
                        isl = slice(i * P, (i + 1) * P)
                        sps_e = pssc.tile([P, 512], f32, tag="sps")
                        nc.tensor.matmul(
                            sps_e, kT[0:HD, isl], qT[0:HD, qsl],
                            start=True, stop=True,
                            tile_position=(0, 0))
                        sps_o = pssc.tile([P, 512], f32, tag="sps")
                        nc.tensor.matmul(
                            sps_o, kT[HD:P, isl], qT[HD:P, qsl],
                            start=True, stop=True,
                            tile_position=(HD, 0))
                        for sps, vaug, ctx in (
                            (sps_e, vaug_e, ctx_e),
                            (sps_o, vaug_o, ctx_o),
                        ):
                            pt = ptp.tile([P, 512], bf16, tag="pt")
                            jd = i - qc * 4  # diag col subtile
                            if jd < 0:  # fully visible
                                nc.scalar.activation(pt, sps, AF.Exp)
                            else:
                                if jd > 0:
                                    nc.vector.memset(
                                        pt[:, 0:jd * P].bitcast(f32),
                                        0.0)
                                dsl = slice(jd * P, (jd + 1) * P)
                                nc.scalar.activation(
                                    pt[:, dsl], sps[:, dsl], AF.Exp)
                                nc.vector.tensor_mul(
                                    pt[:, dsl], pt[:, dsl], tril)
                                if jd < 3:
                                    rsl = slice((jd + 1) * P, 512)
                                    nc.scalar.activation(
                                        pt[:, rsl], sps[:, rsl],
                                        AF.Exp)
                            nc.tensor.matmul(
                                ctx, vaug[:, i, :], pt,
                                start=(n == 0),
                                stop=(n == len(kv_tiles) - 1))
                    # softmax normalization per head
                    for hh, ctx in ((0, ctx_e), (1, ctx_o)):
                        rr = ptp.tile([HD + 1, 512], f32, tag="rr")
                        rr0 = ptp.tile([1, 512], f32, tag="rr0")
                        bc = ptp.tile([HD, 512], f32, tag="bc")
                        nc.vector.reciprocal(
                            rr[HD:HD + 1, :], ctx[HD:HD + 1, :])
                        nc.sync.dma_start(rr0, rr[HD:HD + 1, :])
                        nc.gpsimd.partition_broadcast(bc, rr0)
                        if hh == 0:
                            nc.vector.tensor_mul(
                                ctxT[0:HD, hp, qsl], ctx[0:HD, :], bc)
                        else:
                            tmp = ptp.tile([HD, 512], f32r, tag="tmp")
                            nc.vector.tensor_mul(tmp, ctx[0:HD, :], bc)
                            nc.sync.dma_start(ctxT[HD:P, hp, qsl], tmp)

                # partial Wo + 0.5*(x+bo) for this chunk's 4 row tiles
                for qt in range(4 * qc, 4 * qc + 4):
                    xo_t = ph3.tile([P, D], f32, tag="xo_t")
                    nc.sync.dma_start(xo_t, xl_d[qt * P:(qt + 1) * P, :])
                    x2_t = ph3.tile([P, D], bf16, tag="x2_t")
                    for dc in range(2):
                        dsl = slice(dc * 512, (dc + 1) * 512)
                        acc = ps3.tile([P, 512], f32, tag="acc")
                        for pc in range(HPL):
                            nc.tensor.matmul(
                                acc, ctxT[:, pc, qt * P:(qt + 1) * P],
                                wo_sb[:, pc, dsl],
                                start=(pc == 0), stop=(pc == HPL - 1))
                        nc.vector.scalar_tensor_tensor(
                            out=x2_t[:, dsl], in0=xo_t[:, dsl], scalar=0.5,
                            in1=acc, op0=ALU.mult, op1=ALU.add)
                        nc.vector.tensor_add(
                            x2_t[:, dsl], x2_t[:, dsl], bo_bc[:, dsl])
                    nc.sync.dma_start(
                        p2d[(qt % 8) // 2][qt // 8, qt % 2], x2_t)
                # chunks complete pairwise: (qc1,qc3) -> rows {4..7,12..15}
                # = RS chunks 2,3 (launched mid-attention); (qc0,qc2) ->
                # RS chunks 0,1 (hidden under LN2/FFN of half 1)
                if qc == 3:
                    for k in (2, 3):
                        nc.gpsimd.collective_compute(
                            "ReduceScatter", ALU.add,
                            replica_groups=[[0, 1], [2, 3], [4, 5], [6, 7]],
                            ins=[p2d[k].opt()],
                            outs=[x2d[2 * k:2 * k + 2].opt()])
                elif qc == 2:
                    for k in (0, 1):
                        nc.gpsimd.collective_compute(
                            "ReduceScatter", ALU.add,
                            replica_groups=[[0, 1], [2, 3], [4, 5], [6, 7]],
                            ins=[p2d[k].opt()],
                            outs=[x2d[2 * k:2 * k + 2].opt()])

    # ---------- Phase 4: LN2 + transpose (own half) ----------
    with tc.tile_pool(name="h2Tp" + sfx, bufs=1) as h2Tp:
        w1sb = h2Tp.tile([P, FT, DT, P], fp8, tag="w1sb")
        nc.sync.dma_start(
            w1sb, w1_d.ap().rearrange("fc p dt m -> p fc (dt m)"))
        w2sb = h2Tp.tile([P, FT // 2, 2, D], fp8, tag="w2sb")
        nc.sync.dma_start(
            w2sb, w2_d.ap().rearrange("fc p i n -> p fc (i n)"))
        h2Tc = []
        for i in range(2):
            h2T_i = h2Tp.tile([P, DT, 512], fp8, tag=f"h2T{i}")
            h2Tc.append(h2T_i)
        with (
            tc.tile_pool(name="ln2" + sfx, bufs=5) as ln2,
            tc.tile_pool(name="ps4" + sfx, bufs=4, space="PSUM") as ps4,
        ):
            for qt in (4, 5, 6, 7, 0, 1, 2, 3):
                x2_t = ln2.tile([P, D], bf16, tag="x2_t")
                nc.sync.dma_start(x2_t, x2d[qt])
                st = ln2.tile([P, 2, 6], f32, tag="st")
                nc.vector.bn_stats(st[:, 0, :], x2_t[:, 0:512])
                nc.vector.bn_stats(st[:, 1, :], x2_t[:, 512:1024])
                mv = ln2.tile([P, 2], f32, tag="mv")
                nc.vector.bn_aggr(mv, st)
                rstd = ln2.tile([P, 1], f32, tag="rstd")
                nc.scalar.activation(rstd, mv[:, 1:2], AF.Sqrt, bias=eps)
                nc.vector.reciprocal(rstd, rstd)
                nb = ln2.tile([P, 2], f32, tag="nb")
                nc.vector.tensor_scalar_mul(nb[:, 0:1], rstd, -1.0)
                nc.vector.tensor_mul(nb[:, 1:2], mv[:, 0:1], nb[:, 0:1])
                h2_t = ln2.tile([P, D], f32r, tag="h2_t")
                qpos = qt % 4
                for dh in range(2):
                    hsl = slice(dh * 512, (dh + 1) * 512)
                    nc.scalar.activation(
                        h2_t[:, hsl], x2_t[:, hsl], AF.Identity,
                        bias=nb[:, 1:2], scale=rstd)
                    tp = ps4.tile([P, 4, P], f32r, tag="tp")
                    for k in range(4):
                        dt = dh * 4 + k
                        nc.tensor.transpose(
                            tp[:, k, :],
                            h2_t[:, dt * P:(dt + 1) * P], ident)
                    dst = h2Tc[qt // 4][:, dh * 4:dh * 4 + 4,
                                        qpos * P:(qpos + 1) * P]
                    if dh == 0:
                        nc.scalar.copy(dst, tp)  # f32r -> fp8 convert
                    else:
                        nc.vector.tensor_copy(dst, tp)

        # ---------- Phase 5: FFN (fp8 DoubleRow) ----------
        with (
            tc.tile_pool(name="ffcst" + sfx, bufs=1) as ffcp,
            tc.tile_pool(name="ffw" + sfx, bufs=5) as ffw,
            tc.tile_pool(name="g1p" + sfx, bufs=1) as g1p,
            tc.tile_pool(name="ffo" + sfx, bufs=3) as ffo,
            tc.tile_pool(name="psa" + sfx, bufs=3, space="PSUM") as psa,
            tc.tile_pool(name="psf" + sfx, bufs=1, space="PSUM") as psf,
        ):
            b1f_sb = ffcp.tile([P, FT], f32)
            nc.sync.dma_start(b1f_sb, b1f_d[:, :])
            b2_bc = ffcp.tile([P, D], f32)
            nc.sync.dma_start(b2_bc, b2_d.ap().to_broadcast([P, D]))
            g1 = g1p.tile([P, FT, 512], fp8)
            for qc in (1, 0):
                qsl = slice(qc * 512, (qc + 1) * 512)
                # W1 + GELU for this q chunk, all ff chunks
                for fc in range(FT):
                    aps = psa.tile([P, 512], f32, tag="aps")
                    for dt in range(0, DT, 2):
                        nc.tensor.matmul(
                            aps, w1sb[:, fc, dt:dt + 2, :],
                            h2Tc[qc][:, dt:dt + 2, :],
                            start=(dt == 0), stop=(dt == DT - 2),
                            perf_mode=DR)
                    nc.scalar.activation(
                        g1[:, fc, :], aps, AF.Gelu,
                        bias=b1f_sb[:, fc:fc + 1], scale=1.0 / W1S)
                # W2 for this q chunk
                for dh in range(2):
                    dsl = slice(dh * 512, (dh + 1) * 512)
                    fps = []
                    for j in range(4):
                        fps_j = psf.tile([P, 512], f32, tag=f"fps{j}")
                        fps.append(fps_j)
                    for fc in range(FT // 2):
                        for j in range(4):
                            nc.tensor.matmul(
                                fps[j],
                                g1[:, 2 * fc:2 * fc + 2, j * P:(j + 1) * P],
                                w2sb[:, fc, :, dsl], start=(fc == 0),
                                stop=(fc == FT // 2 - 1),
                                perf_mode=DR)
                    for j in range(4):
                        qt = qc * 4 + j
                        o_t = ffo.tile([P, 512], f32, tag="o_t")
                        x2s = ffo.tile([P, 512], bf16, tag="x2s")
                        nc.sync.dma_start(x2s, x2d[qt, :, dsl])
                        nc.vector.scalar_tensor_tensor(
                            out=o_t, in0=fps[j], scalar=1.0 / W2S,
                            in1=x2s, op0=ALU.mult, op1=ALU.add)
                        nc.vector.tensor_add(o_t, o_t, b2_bc[:, dsl])
                        nc.sync.dma_start(
                            out_d[qt * P:(qt + 1) * P, dsl], o_t)


def _build_program(reps=1):
    nc = bacc.Bacc(None, target_bir_lowering=False)

    xl_d = nc.dram_tensor("xl", (T, D), f32, kind="ExternalInput")
    wq_d = nc.dram_tensor("wq", (D, HL * HD), f32r, kind="ExternalInput")
    wk_d = nc.dram_tensor("wk", (D, HL * HD), f32r, kind="ExternalInput")
    wv_d = nc.dram_tensor("wv", (D, HL * HD), f32r, kind="ExternalInput")
    wo_d = nc.dram_tensor("wo", (HL * HD, D), f32r, kind="ExternalInput")
    # w1: [fc, p, dt, m] pre-arranged fp8 (scaled by W1S)
    w1_d = nc.dram_tensor("w1", (FF // P, P, D // P, P), fp8,
                          kind="ExternalInput")
    # w2: [fc2, p, i, n] pre-arranged fp8 (scaled by W2S), i = k-pair
    w2_d = nc.dram_tensor("w2", (FF // (2 * P), P, 2, D), fp8,
                          kind="ExternalInput")
    qkvb_d = nc.dram_tensor("qkvb", (P, 3 * HPL), f32, kind="ExternalInput")
    bo_d = nc.dram_tensor("bo_", (1, D), f32, kind="ExternalInput")
    b1f_d = nc.dram_tensor("b1f", (P, FF // P), f32, kind="ExternalInput")
    b2_d = nc.dram_tensor("b2_", (1, D), f32, kind="ExternalInput")
    out_d = nc.dram_tensor("out", (TQ, D), f32, kind="ExternalOutput")

    NQ = TQ // P
    NT = T // P

    with tile.TileContext(nc) as tc:
        with (
            tc.tile_pool(name="const", bufs=1) as const,
            tc.tile_pool(name="dramp", bufs=1, space="DRAM") as dramp,
        ):
            ident_f = const.tile([P, P], f32)
            make_identity(nc, ident_f)
            ident = const.tile([P, P], f32r)
            nc.vector.tensor_copy(ident, ident_f)
            ident_b = const.tile([P, P], bf16)
            nc.vector.tensor_copy(ident_b, ident_f)
            # S^T-space causal keep mask: keep where kv(part) <= q(free)
            tril_f = const.tile([P, P], f32)
            nc.gpsimd.memset(tril_f, 1.0)
            nc.gpsimd.affine_select(
                out=tril_f, in_=tril_f, compare_op=ALU.is_ge, fill=0.0,
                base=0, pattern=[[1, P]], channel_multiplier=-1,
            )
            tril = const.tile([P, P], f32r)
            nc.vector.tensor_copy(tril, tril_f)
            ones16 = const.tile([P, NT], f32)
            nc.vector.memset(ones16, 1.0)
            qkvb = const.tile([P, 3 * HPL], f32)
            nc.sync.dma_start(qkvb, qkvb_d[:, :])
            eps = const.tile([P, 1], f32)
            nc.vector.memset(eps, 1e-5)

            x2d = dramp.tile([NQ, P, D], bf16)  # post-RS own-half residual
            # partial attn out (bf16), one contiguous [half, tile] buffer
            # per ReduceScatter chunk of two row-tiles
            p2d = []
            for k in range(4):
                p2_k = dramp.tile([2, 2, P, D], bf16, tag=f"p2{k}")
                p2d.append(p2_k)

            cst = (ident, ident_b, tril, ones16, qkvb, eps)
            dram = (xl_d, wq_d, wk_d, wv_d, wo_d, w1_d, w2_d,
                    bo_d, b1f_d, b2_d, out_d)
            for rep in range(reps):
                sfx = f"r{rep}" if reps > 1 else ""
                _emit_body(nc, tc, sfx, cst, x2d, p2d, dram)

    nc.compile()
    return nc


def _prep_inputs(inputs):
    """Host-side: fold LN affine + score scale into weights; build per-core maps."""
    x = np.asarray(inputs["x"], dtype=np.float32)
    g1, b1_ = np.asarray(inputs["ln1_g"], np.float32), np.asarray(inputs["ln1_b"], np.float32)
    g2, b2_ = np.asarray(inputs["ln2_g"], np.float32), np.asarray(inputs["ln2_b"], np.float32)
    Wq = np.asarray(inputs["Wq"], np.float32)  # [H, D, HD]
    Wk = np.asarray(inputs["Wk"], np.float32)
    Wv = np.asarray(inputs["Wv"], np.float32)
    bq = np.asarray(inputs["bq"], np.float32)  # [H, HD]
    bk = np.asarray(inputs["bk"], np.float32)
    bv = np.asarray(inputs["bv"], np.float32)
    Wo = np.asarray(inputs["Wo"], np.float32)
    bo = np.asarray(inputs["bo"], np.float32)
    W1 = np.asarray(inputs["W1"], np.float32)
    b1 = np.asarray(inputs["b1"], np.float32)
    W2 = np.asarray(inputs["W2"], np.float32)
    b2 = np.asarray(inputs["b2"], np.float32)

    sc = 1.0 / np.sqrt(np.float32(HD))
    # [H, D, HD] -> [D, H*HD]
    wq_flat = np.transpose(Wq, (1, 0, 2)).reshape(D, D)
    wk_flat = np.transpose(Wk, (1, 0, 2)).reshape(D, D)
    wv_flat = np.transpose(Wv, (1, 0, 2)).reshape(D, D)
    wq_f = (g1[:, None] * wq_flat) * sc
    wk_f = g1[:, None] * wk_flat
    wv_f = g1[:, None] * wv_flat
    bq_f = (b1_ @ wq_flat + bq.reshape(D)) * sc
    bk_f = b1_ @ wk_flat + bk.reshape(D)
    bv_f = b1_ @ wv_flat + bv.reshape(D)

    w1_f = g2[:, None] * W1
    b1_f = (b2_ @ W1 + b1).reshape(FF // P, P).T.copy()  # [P, FF//P]

    import ml_dtypes

    e4m3 = ml_dtypes.float8_e4m3fn
    DT_, FT_ = D // P, FF // P
    # [fc, p, dt, m] layout; scaled into fp8's normal range
    w1_8 = np.clip(w1_f * W1S, -240, 240).astype(e4m3)
    w1_8 = w1_8.reshape(DT_, P, FT_, P).transpose(2, 1, 0, 3).copy()
    w2_8 = np.clip(W2 * W2S, -240, 240).astype(e4m3)
    w2_8 = w2_8.reshape(FT_ // 2, 2, P, D).transpose(0, 2, 1, 3).copy()

    shared = {
        "w1": w1_8, "w2": w2_8,
        "bo_": np.ascontiguousarray(0.5 * bo.reshape(1, D)),
        "b1f": np.ascontiguousarray(b1_f), "b2_": b2.reshape(1, D),
    }
    # per head-half: weight column/row slices + bias table
    half = {}
    for o in range(2):
        hsl = slice(o * HL * HD, (o + 1) * HL * HD)
        qkvb = np.zeros((P, 3 * HPL), np.float32)
        for hp in range(HPL):
            gp = o * HPL + hp  # global head pair
            for wi, bf in enumerate((bq_f, bk_f, bv_f)):
                qkvb[0:HD, 3 * hp + wi] = bf[(2 * gp) * HD:(2 * gp + 1) * HD]
                qkvb[HD:P, 3 * hp + wi] = bf[(2 * gp + 1) * HD:(2 * gp + 2) * HD]
        half[o] = {
            "wq": np.ascontiguousarray(wq_f[:, hsl]),
            "wk": np.ascontiguousarray(wk_f[:, hsl]),
            "wv": np.ascontiguousarray(wv_f[:, hsl]),
            "wo": np.ascontiguousarray(Wo[hsl, :]),
            "qkvb": qkvb,
        }
    in_maps = []
    for c in range(8):
        b, o = c // 2, c % 2
        m = dict(shared)
        m.update(half[o])
        m["xl"] = np.ascontiguousarray(x[b])
        in_maps.append(m)
    return in_maps


def kernel(**inputs):
    if "nc" not in _CACHE:
        _CACHE["nc"] = _build_program()
    nc = _CACHE["nc"]
    in_maps = _prep_inputs(inputs)
    res = run_bass_kernel_spmd(nc, in_maps, core_ids=list(range(8)))
    out = np.empty((B, T, D), np.float32)
    for c in range(8):
        b, o = c // 2, c % 2
        out[b, o * TQ:(o + 1) * TQ] = res.results[c]["out"]
    return out


# revision 46
# speedup vs baseline: 1.0756x; 1.0756x over previous
"""Fused transformer block (nn_Block_2388001816768) on 8 Trainium2 NeuronCores.

Sharding: (batch, head-half) -> one core. Core c handles batch c//2 and
heads [8o, 8o+8) where o = c%2, over the FULL sequence. Causal attention
is exact (no masked-tile waste): q-chunk qc attends kv tiles 0..4qc+3
with a tril constant on the diagonal tile.

After the Wo projection each core holds a partial attention output
(its 8 heads' contribution) plus 0.5*(x + bo); a pairwise ReduceScatter
(add) between the two cores of a batch yields x2 = x + attn_out, split
so each core keeps its sequence half for LN2 + FFN.

Large matmuls run in float32r (TF32-like, full PE rate at free>=256),
fp32 accum. The FFN runs fp8e4 DoubleRow (2x PE rate): W1*16 / W2*64
are pre-scaled into fp8's normal range host-side; the inverse scales
fold into the GELU input scale and the output epilogue. LN scale/shift
and the 1/sqrt(HD) score scale are folded into projection weights
host-side.
"""

import numpy as np

import concourse.bacc as bacc
import concourse.bass as bass  # noqa: F401
import concourse.mybir as mybir
import concourse.tile as tile
from concourse.bass_utils import run_bass_kernel_spmd
from concourse.masks import make_identity

B, T, D, H = 4, 2048, 1024, 16
HD = D // H  # 64
FF = 4 * D  # 4096
TQ = T // 2  # output rows per core = 1024
P = 128
HL = H // 2  # heads per core = 8
HPL = HL // 2  # head pairs per core = 4

f32 = mybir.dt.float32
f32r = mybir.dt.float32r
bf16 = mybir.dt.bfloat16
fp8 = mybir.dt.float8e4
AF = mybir.ActivationFunctionType
ALU = mybir.AluOpType
DR = mybir.MatmulPerfMode.DoubleRow
W1S = 16.0  # host-side weight scale (fp8 range), undone by activation scale
W2S = 64.0

_CACHE = {}


def _emit_body(nc, tc, sfx, cst, x2d, p2d, dram):
    (xl_d, wq_d, wk_d, wv_d, wo_d, w1_d, w2_d, bo_d, b1f_d, b2_d, out_d) = dram
    ident, ident_b, tril, ones16, qkvb, eps = cst

    DT = D // P  # 8 d-tiles
    NT = T // P  # 16 t-tiles
    NQ = TQ // P  # 8 own-half q-tiles
    FT = FF // P  # 32 ff-tiles

    with tc.tile_pool(name="ctxp" + sfx, bufs=1) as ctxp:
        ctxT = ctxp.tile([P, HPL, T], bf16)  # ctx^T head-pair-stacked

        # ---------- Phases 1+2: LN1, then per head pair QKV followed
        # immediately by attention for q-chunks 1 and 3 — the exp stream
        # (Activation) overlaps the next pair's projections (PE). Chunks
        # 0 and 2 run after the first ReduceScatter pair launches.
        with (
            tc.tile_pool(name="whead" + sfx, bufs=1) as whead,
            tc.tile_pool(name="vcp" + sfx, bufs=2) as vcp,
            tc.tile_pool(name="ptq" + sfx, bufs=3) as ptq,
            tc.tile_pool(name="nrm" + sfx, bufs=1) as nrm,
            tc.tile_pool(name="pvp" + sfx, bufs=1, space="PSUM") as pvp,
            tc.tile_pool(name="pssc" + sfx, bufs=3, space="PSUM") as pssc,
            tc.tile_pool(name="psctx" + sfx, bufs=1, space="PSUM") as psctx,
        ):
            qTs, kTs, vaugs = [], [], []

            def qkv(hp, hTc):
                wp = whead.tile([P, 3, DT, 2 * HD], f32r, tag="wp")
                for wi, w_dram in enumerate((wq_d, wk_d, wv_d)):
                    nc.sync.dma_start(
                        wp[:, wi],
                        w_dram[:, hp * 2 * HD:(hp + 1) * 2 * HD]
                        .rearrange("(dt q) m -> q dt m", q=P))
                qT = ctxp.tile([P, T], f32r, tag=f"qT{hp}")
                kT = ctxp.tile([P, T], f32r, tag=f"kT{hp}")
                vaug_e = ctxp.tile([P, NT, HD + 1], bf16, tag=f"va{hp}e")
                vaug_o = ctxp.tile([P, NT, HD + 1], bf16, tag=f"va{hp}o")
                qTs.append(qT)
                kTs.append(kT)
                vaugs.append((vaug_e, vaug_o))
                nc.vector.tensor_copy(
                    vaug_e[:, :, HD:HD + 1], ones16.unsqueeze(2))
                nc.vector.tensor_copy(
                    vaug_o[:, :, HD:HD + 1], ones16.unsqueeze(2))
                for (wi, bcol) in ((0, 3 * hp), (1, 3 * hp + 1),
                                   (2, 3 * hp + 2)):
                    for c in range(T // 512):
                        pp = pssc.tile([P, 512], f32, tag="sps")
                        for dt in range(DT):
                            nc.tensor.matmul(
                                pp, wp[:, wi, dt, :],
                                hTc[c][:, dt, :],
                                start=(dt == 0), stop=(dt == DT - 1))
                        csl = slice(c * 512, (c + 1) * 512)
                        if wi == 0:
                            nc.vector.tensor_scalar_add(
                                out=qT[:, csl], in0=pp,
                                scalar1=qkvb[:, bcol:bcol + 1])
                        elif wi == 1:
                            nc.vector.tensor_scalar_add(
                                out=kT[:, csl], in0=pp,
                                scalar1=qkvb[:, bcol:bcol + 1])
                        else:
                            vc = vcp.tile([P, 512], bf16, tag="vc")
                            nc.vector.tensor_scalar_add(
                                out=vc, in0=pp,
                                scalar1=qkvb[:, bcol:bcol + 1])
                            for k in range(4):
                                kt = 4 * c + k
                                vp = pvp.tile([P, P], bf16, tag="vp")
                                nc.tensor.transpose(
                                    vp, vc[:, k * P:(k + 1) * P],
                                    ident_b)
                                nc.vector.tensor_copy(
                                    vaug_e[:, kt, 0:HD], vp[:, 0:HD])
                                nc.vector.tensor_copy(
                                    vaug_o[:, kt, 0:HD], vp[:, HD:P])

            def attn(hp, qc):
                qsl = slice(qc * 512, (qc + 1) * 512)
                kv_tiles = list(range(0, (qc + 1) * 4))
                qT, kT = qTs[hp], kTs[hp]
                vaug_e, vaug_o = vaugs[hp]
                ctx_e = psctx.tile([HD + 1, 512], f32, tag="ctx_e")
                ctx_o = psctx.tile([HD + 1, 512], f32, tag="ctx_o")
                for n, i in enumerate(kv_tiles):
                    isl = slice(i * P, (i + 1) * P)
                    sps_e = pssc.tile([P, 512], f32, tag="sps")
                    nc.tensor.matmul(
                        sps_e, kT[0:HD, isl], qT[0:HD, qsl],
                        start=True, stop=True, tile_position=(0, 0))
                    sps_o = pssc.tile([P, 512], f32, tag="sps")
                    nc.tensor.matmul(
                        sps_o, kT[HD:P, isl], qT[HD:P, qsl],
                        start=True, stop=True, tile_position=(HD, 0))
                    for sps, vaug, ctx in (
                        (sps_e, vaug_e, ctx_e),
                        (sps_o, vaug_o, ctx_o),
                    ):
                        pt = ptq.tile([P, 512], bf16, tag="pt")
                        jd = i - qc * 4  # diag col subtile
                        if jd < 0:  # fully visible
                            nc.scalar.activation(pt, sps, AF.Exp)
                        else:
                            if jd > 0:
                                nc.vector.memset(
                                    pt[:, 0:jd * P].bitcast(f32), 0.0)
                            dsl = slice(jd * P, (jd + 1) * P)
                            nc.scalar.activation(
                                pt[:, dsl], sps[:, dsl], AF.Exp)
                            nc.vector.tensor_mul(
                                pt[:, dsl], pt[:, dsl], tril)
                            if jd < 3:
                                rsl = slice((jd + 1) * P, 512)
                                nc.scalar.activation(
                                    pt[:, rsl], sps[:, rsl], AF.Exp)
                        nc.tensor.matmul(
                            ctx, vaug[:, i, :], pt,
                            start=(n == 0),
                            stop=(n == len(kv_tiles) - 1))
                # softmax normalization per head
                for hh, ctx in ((0, ctx_e), (1, ctx_o)):
                    rr = nrm.tile([HD + 1, 512], f32, tag="rr")
                    rr0 = nrm.tile([1, 512], f32, tag="rr0")
                    bc = nrm.tile([HD, 512], f32, tag="bc")
                    nc.vector.reciprocal(
                        rr[HD:HD + 1, :], ctx[HD:HD + 1, :])
                    nc.sync.dma_start(rr0, rr[HD:HD + 1, :])
                    nc.gpsimd.partition_broadcast(bc, rr0)
                    if hh == 0:
                        nc.vector.tensor_mul(
                            ctxT[0:HD, hp, qsl], ctx[0:HD, :], bc)
                    else:
                        tmp = nrm.tile([HD, 512], bf16, tag="tmp")
                        nc.vector.tensor_mul(tmp, ctx[0:HD, :], bc)
                        nc.sync.dma_start(ctxT[HD:P, hp, qsl], tmp)

            with tc.tile_pool(name="hTp" + sfx, bufs=1) as hTp:
                # h^T in 4 t-chunks of 512 so QKV can overlap phase 1
                hTc = []
                for i in range(4):
                    hT_i = hTp.tile([P, DT, 512], f32r, tag=f"hT{i}")
                    hTc.append(hT_i)

                # ---------- Phase 1: LN1 + transpose (full T) ----------
                with (
                    tc.tile_pool(name="ln1" + sfx, bufs=2) as ln1,
                    tc.tile_pool(name="ps1" + sfx, bufs=2, space="PSUM") as ps1,
                ):
                    for tt in range(NT):
                        x_t = ln1.tile([P, D], f32, tag="x_t")
                        nc.sync.dma_start(x_t, xl_d[tt * P:(tt + 1) * P, :])
                        st = ln1.tile([P, 2, 6], f32, tag="st")
                        nc.vector.bn_stats(st[:, 0, :], x_t[:, 0:512])
                        nc.vector.bn_stats(st[:, 1, :], x_t[:, 512:1024])
                        mv = ln1.tile([P, 2], f32, tag="mv")
                        nc.vector.bn_aggr(mv, st)
                        rstd = ln1.tile([P, 1], f32, tag="rstd")
                        nc.scalar.activation(rstd, mv[:, 1:2], AF.Sqrt,
                                             bias=eps)
                        nc.vector.reciprocal(rstd, rstd)
                        nb = ln1.tile([P, 2], f32, tag="nb")
                        nc.vector.tensor_scalar_mul(nb[:, 0:1], rstd, -1.0)
                        nc.vector.tensor_mul(
                            nb[:, 1:2], mv[:, 0:1], nb[:, 0:1])
                        h_t = ln1.tile([P, D], f32r, tag="h_t")
                        tpos = tt % 4
                        for dh in range(2):
                            hsl = slice(dh * 512, (dh + 1) * 512)
                            nc.scalar.activation(
                                h_t[:, hsl], x_t[:, hsl], AF.Identity,
                                bias=nb[:, 1:2], scale=rstd)
                            tp = ps1.tile([P, 4, P], f32r, tag="tp")
                            for k in range(4):
                                dt = dh * 4 + k
                                nc.tensor.transpose(
                                    tp[:, k, :],
                                    h_t[:, dt * P:(dt + 1) * P], ident)
                            dst = hTc[tt // 4][:, dh * 4:dh * 4 + 4,
                                               tpos * P:(tpos + 1) * P]
                            if dh == 0:
                                nc.scalar.copy(dst, tp)
                            else:
                                nc.vector.tensor_copy(dst, tp)

                # ---------- Phase 2a: QKV + attn chunks 1, 3 ----------
                for hp in range(HPL):
                    qkv(hp, hTc)
                    attn(hp, 1)
                    attn(hp, 3)

            # ---------- Phase 2b: Wo + RS, then attn chunks 0, 2 ----------
            with (
                tc.tile_pool(name="wop" + sfx, bufs=1) as wop,
                tc.tile_pool(name="ph3" + sfx, bufs=3) as ph3,
            ):
                wo_sb = wop.tile([P, HPL, D], bf16)
                nc.sync.dma_start(
                    wo_sb, wo_d.ap().rearrange("(pc p) n -> p pc n", p=P))
                bo_bc = wop.tile([P, D], f32)  # pre-halved host-side
                nc.sync.dma_start(bo_bc, bo_d.ap().to_broadcast([P, D]))

                def wo_chunk(qc):
                    for qt in range(4 * qc, 4 * qc + 4):
                        xo_t = ph3.tile([P, D], f32, tag="xo_t")
                        nc.sync.dma_start(
                            xo_t, xl_d[qt * P:(qt + 1) * P, :])
                        x2_t = ph3.tile([P, D], bf16, tag="x2_t")
                        for dc in range(2):
                            dsl = slice(dc * 512, (dc + 1) * 512)
                            acc = pssc.tile([P, 512], f32, tag="sps")
                            for pc in range(HPL):
                                nc.tensor.matmul(
                                    acc, ctxT[:, pc, qt * P:(qt + 1) * P],
                                    wo_sb[:, pc, dsl],
                                    start=(pc == 0), stop=(pc == HPL - 1))
                            nc.vector.scalar_tensor_tensor(
                                out=x2_t[:, dsl], in0=xo_t[:, dsl],
                                scalar=0.5, in1=acc,
                                op0=ALU.mult, op1=ALU.add)
                            nc.vector.tensor_add(
                                x2_t[:, dsl], x2_t[:, dsl], bo_bc[:, dsl])
                        nc.sync.dma_start(
                            p2d[(qt % 8) // 2][qt // 8, qt % 2], x2_t)

                def rs(k):
                    nc.gpsimd.collective_compute(
                        "ReduceScatter", ALU.add,
                        replica_groups=[[0, 1], [2, 3], [4, 5], [6, 7]],
                        ins=[p2d[k].opt()],
                        outs=[x2d[2 * k:2 * k + 2].opt()])

                wo_chunk(1)
                wo_chunk(3)
                rs(2)
                rs(3)
                for hp in range(HPL):
                    attn(hp, 0)
                    attn(hp, 2)
                wo_chunk(0)
                wo_chunk(2)
                rs(0)
                rs(1)

    # ---------- Phase 4: LN2 + transpose (own half) ----------
    with tc.tile_pool(name="h2Tp" + sfx, bufs=1) as h2Tp:
        w1sb = h2Tp.tile([P, FT, DT, P], fp8, tag="w1sb")
        nc.sync.dma_start(
            w1sb, w1_d.ap().rearrange("fc p dt m -> p fc (dt m)"))
        w2sb = h2Tp.tile([P, FT // 2, 2, D], fp8, tag="w2sb")
        nc.sync.dma_start(
            w2sb, w2_d.ap().rearrange("fc p i n -> p fc (i n)"))
        h2Tc = []
        for i in range(2):
            h2T_i = h2Tp.tile([P, DT, 512], fp8, tag=f"h2T{i}")
            h2Tc.append(h2T_i)
        with (
            tc.tile_pool(name="ln2" + sfx, bufs=5) as ln2,
            tc.tile_pool(name="ps4" + sfx, bufs=4, space="PSUM") as ps4,
        ):
            for qt in (4, 5, 6, 7, 0, 1, 2, 3):
                x2_t = ln2.tile([P, D], bf16, tag="x2_t")
                nc.sync.dma_start(x2_t, x2d[qt])
                st = ln2.tile([P, 2, 6], f32, tag="st")
                nc.vector.bn_stats(st[:, 0, :], x2_t[:, 0:512])
                nc.vector.bn_stats(st[:, 1, :], x2_t[:, 512:1024])
                mv = ln2.tile([P, 2], f32, tag="mv")
                nc.vector.bn_aggr(mv, st)
                rstd = ln2.tile([P, 1], f32, tag="rstd")
                nc.scalar.activation(rstd, mv[:, 1:2], AF.Sqrt, bias=eps)
                nc.vector.reciprocal(rstd, rstd)
                nb = ln2.tile([P, 2], f32, tag="nb")
                nc.vector.tensor_scalar_mul(nb[:, 0:1], rstd, -1.0)
                nc.vector.tensor_mul(nb[:, 1:2], mv[:, 0:1], nb[:, 0:1])
                h2_t = ln2.tile([P, D], f32r, tag="h2_t")
                qpos = qt % 4
                for dh in range(2):
                    hsl = slice(dh * 512, (dh + 1) * 512)
                    nc.scalar.activation(
                        h2_t[:, hsl], x2_t[:, hsl], AF.Identity,
                        bias=nb[:, 1:2], scale=rstd)
                    tp = ps4.tile([P, 4, P], f32r, tag="tp")
                    for k in range(4):
                        dt = dh * 4 + k
                        nc.tensor.transpose(
                            tp[:, k, :],
                            h2_t[:, dt * P:(dt + 1) * P], ident)
                    dst = h2Tc[qt // 4][:, dh * 4:dh * 4 + 4,
                                        qpos * P:(qpos + 1) * P]
                    if dh == 0:
                        nc.scalar.copy(dst, tp)  # f32r -> fp8 convert
                    else:
                        nc.vector.tensor_copy(dst, tp)

        # ---------- Phase 5: FFN (fp8 DoubleRow) ----------
        with (
            tc.tile_pool(name="ffcst" + sfx, bufs=1) as ffcp,
            tc.tile_pool(name="ffw" + sfx, bufs=5) as ffw,
            tc.tile_pool(name="g1p" + sfx, bufs=1) as g1p,
            tc.tile_pool(name="ffo" + sfx, bufs=3) as ffo,
            tc.tile_pool(name="psa" + sfx, bufs=3, space="PSUM") as psa,
            tc.tile_pool(name="psf" + sfx, bufs=1, space="PSUM") as psf,
        ):
            b1f_sb = ffcp.tile([P, FT], f32)
            nc.sync.dma_start(b1f_sb, b1f_d[:, :])
            b2_bc = ffcp.tile([P, D], f32)
            nc.sync.dma_start(b2_bc, b2_d.ap().to_broadcast([P, D]))
            g1 = g1p.tile([P, FT, 512], fp8)
            for qc in (1, 0):
                qsl = slice(qc * 512, (qc + 1) * 512)
                # W1 + GELU for this q chunk, all ff chunks
                for fc in range(FT):
                    aps = psa.tile([P, 512], f32, tag="aps")
                    for dt in range(0, DT, 2):
                        nc.tensor.matmul(
                            aps, w1sb[:, fc, dt:dt + 2, :],
                            h2Tc[qc][:, dt:dt + 2, :],
                            start=(dt == 0), stop=(dt == DT - 2),
                            perf_mode=DR)
                    nc.scalar.activation(
                        g1[:, fc, :], aps, AF.Gelu,
                        bias=b1f_sb[:, fc:fc + 1], scale=1.0 / W1S)
                # W2 for this q chunk
                for dh in range(2):
                    dsl = slice(dh * 512, (dh + 1) * 512)
                    fps = []
                    for j in range(4):
                        fps_j = psf.tile([P, 512], f32, tag=f"fps{j}")
                        fps.append(fps_j)
                    for fc in range(FT // 2):
                        for j in range(4):
                            nc.tensor.matmul(
                                fps[j],
                                g1[:, 2 * fc:2 * fc + 2, j * P:(j + 1) * P],
                                w2sb[:, fc, :, dsl], start=(fc == 0),
                                stop=(fc == FT // 2 - 1),
                                perf_mode=DR)
                    for j in range(4):
                        qt = qc * 4 + j
                        o_t = ffo.tile([P, 512], f32, tag="o_t")
                        x2s = ffo.tile([P, 512], bf16, tag="x2s")
                        nc.sync.dma_start(x2s, x2d[qt, :, dsl])
                        nc.vector.scalar_tensor_tensor(
                            out=o_t, in0=fps[j], scalar=1.0 / W2S,
                            in1=x2s, op0=ALU.mult, op1=ALU.add)
                        nc.vector.tensor_add(o_t, o_t, b2_bc[:, dsl])
                        nc.sync.dma_start(
                            out_d[qt * P:(qt + 1) * P, dsl], o_t)


def _build_program(reps=1):
    nc = bacc.Bacc(None, target_bir_lowering=False)

    xl_d = nc.dram_tensor("xl", (T, D), f32, kind="ExternalInput")
    wq_d = nc.dram_tensor("wq", (D, HL * HD), f32r, kind="ExternalInput")
    wk_d = nc.dram_tensor("wk", (D, HL * HD), f32r, kind="ExternalInput")
    wv_d = nc.dram_tensor("wv", (D, HL * HD), f32r, kind="ExternalInput")
    wo_d = nc.dram_tensor("wo", (HL * HD, D), bf16, kind="ExternalInput")
    # w1: [fc, p, dt, m] pre-arranged fp8 (scaled by W1S)
    w1_d = nc.dram_tensor("w1", (FF // P, P, D // P, P), fp8,
                          kind="ExternalInput")
    # w2: [fc2, p, i, n] pre-arranged fp8 (scaled by W2S), i = k-pair
    w2_d = nc.dram_tensor("w2", (FF // (2 * P), P, 2, D), fp8,
                          kind="ExternalInput")
    qkvb_d = nc.dram_tensor("qkvb", (P, 3 * HPL), f32, kind="ExternalInput")
    bo_d = nc.dram_tensor("bo_", (1, D), f32, kind="ExternalInput")
    b1f_d = nc.dram_tensor("b1f", (P, FF // P), f32, kind="ExternalInput")
    b2_d = nc.dram_tensor("b2_", (1, D), f32, kind="ExternalInput")
    out_d = nc.dram_tensor("out", (TQ, D), f32, kind="ExternalOutput")

    NQ = TQ // P
    NT = T // P

    with tile.TileContext(nc) as tc:
        with (
            tc.tile_pool(name="const", bufs=1) as const,
            tc.tile_pool(name="dramp", bufs=1, space="DRAM") as dramp,
        ):
            ident_f = const.tile([P, P], f32)
            make_identity(nc, ident_f)
            ident = const.tile([P, P], f32r)
            nc.vector.tensor_copy(ident, ident_f)
            ident_b = const.tile([P, P], bf16)
            nc.vector.tensor_copy(ident_b, ident_f)
            # S^T-space causal keep mask: keep where kv(part) <= q(free)
            tril_f = const.tile([P, P], f32)
            nc.gpsimd.memset(tril_f, 1.0)
            nc.gpsimd.affine_select(
                out=tril_f, in_=tril_f, compare_op=ALU.is_ge, fill=0.0,
                base=0, pattern=[[1, P]], channel_multiplier=-1,
            )
            tril = const.tile([P, P], f32r)
            nc.vector.tensor_copy(tril, tril_f)
            ones16 = const.tile([P, NT], f32)
            nc.vector.memset(ones16, 1.0)
            qkvb = const.tile([P, 3 * HPL], f32)
            nc.sync.dma_start(qkvb, qkvb_d[:, :])
            eps = const.tile([P, 1], f32)
            nc.vector.memset(eps, 1e-5)

            x2d = dramp.tile([NQ, P, D], bf16)  # post-RS own-half residual
            # partial attn out (bf16), one contiguous [half, tile] buffer
            # per ReduceScatter chunk of two row-tiles
            p2d = []
            for k in range(4):
                p2_k = dramp.tile([2, 2, P, D], bf16, tag=f"p2{k}")
                p2d.append(p2_k)

            cst = (ident, ident_b, tril, ones16, qkvb, eps)
            dram = (xl_d, wq_d, wk_d, wv_d, wo_d, w1_d, w2_d,
                    bo_d, b1f_d, b2_d, out_d)
            for rep in range(reps):
                sfx = f"r{rep}" if reps > 1 else ""
                _emit_body(nc, tc, sfx, cst, x2d, p2d, dram)

    nc.compile()
    return nc


def _prep_inputs(inputs):
    """Host-side: fold LN affine + score scale into weights; build per-core maps."""
    x = np.asarray(inputs["x"], dtype=np.float32)
    g1, b1_ = np.asarray(inputs["ln1_g"], np.float32), np.asarray(inputs["ln1_b"], np.float32)
    g2, b2_ = np.asarray(inputs["ln2_g"], np.float32), np.asarray(inputs["ln2_b"], np.float32)
    Wq = np.asarray(inputs["Wq"], np.float32)  # [H, D, HD]
    Wk = np.asarray(inputs["Wk"], np.float32)
    Wv = np.asarray(inputs["Wv"], np.float32)
    bq = np.asarray(inputs["bq"], np.float32)  # [H, HD]
    bk = np.asarray(inputs["bk"], np.float32)
    bv = np.asarray(inputs["bv"], np.float32)
    Wo = np.asarray(inputs["Wo"], np.float32)
    bo = np.asarray(inputs["bo"], np.float32)
    W1 = np.asarray(inputs["W1"], np.float32)
    b1 = np.asarray(inputs["b1"], np.float32)
    W2 = np.asarray(inputs["W2"], np.float32)
    b2 = np.asarray(inputs["b2"], np.float32)

    sc = 1.0 / np.sqrt(np.float32(HD))
    # [H, D, HD] -> [D, H*HD]
    wq_flat = np.transpose(Wq, (1, 0, 2)).reshape(D, D)
    wk_flat = np.transpose(Wk, (1, 0, 2)).reshape(D, D)
    wv_flat = np.transpose(Wv, (1, 0, 2)).reshape(D, D)
    wq_f = (g1[:, None] * wq_flat) * sc
    wk_f = g1[:, None] * wk_flat
    wv_f = g1[:, None] * wv_flat
    bq_f = (b1_ @ wq_flat + bq.reshape(D)) * sc
    bk_f = b1_ @ wk_flat + bk.reshape(D)
    bv_f = b1_ @ wv_flat + bv.reshape(D)

    w1_f = g2[:, None] * W1
    b1_f = (b2_ @ W1 + b1).reshape(FF // P, P).T.copy()  # [P, FF//P]

    import ml_dtypes

    e4m3 = ml_dtypes.float8_e4m3fn
    DT_, FT_ = D // P, FF // P
    # [fc, p, dt, m] layout; scaled into fp8's normal range
    w1_8 = np.clip(w1_f * W1S, -240, 240).astype(e4m3)
    w1_8 = w1_8.reshape(DT_, P, FT_, P).transpose(2, 1, 0, 3).copy()
    w2_8 = np.clip(W2 * W2S, -240, 240).astype(e4m3)
    w2_8 = w2_8.reshape(FT_ // 2, 2, P, D).transpose(0, 2, 1, 3).copy()

    shared = {
        "w1": w1_8, "w2": w2_8,
        "bo_": np.ascontiguousarray(0.5 * bo.reshape(1, D)),
        "b1f": np.ascontiguousarray(b1_f), "b2_": b2.reshape(1, D),
    }
    # per head-half: weight column/row slices + bias table
    half = {}
    for o in range(2):
        hsl = slice(o * HL * HD, (o + 1) * HL * HD)
        qkvb = np.zeros((P, 3 * HPL), np.float32)
        for hp in range(HPL):
            gp = o * HPL + hp  # global head pair
            for wi, bf in enumerate((bq_f, bk_f, bv_f)):
                qkvb[0:HD, 3 * hp + wi] = bf[(2 * gp) * HD:(2 * gp + 1) * HD]
                qkvb[HD:P, 3 * hp + wi] = bf[(2 * gp + 1) * HD:(2 * gp + 2) * HD]
        half[o] = {
            "wq": np.ascontiguousarray(wq_f[:, hsl]),
            "wk": np.ascontiguousarray(wk_f[:, hsl]),
            "wv": np.ascontiguousarray(wv_f[:, hsl]),
            "wo": np.ascontiguousarray(Wo[hsl, :]).astype(ml_dtypes.bfloat16),
            "qkvb": qkvb,
        }
    in_maps = []
    for c in range(8):
        b, o = c // 2, c % 2
        m = dict(shared)
        m.update(half[o])
        m["xl"] = np.ascontiguousarray(x[b])
        in_maps.append(m)
    return in_maps


def kernel(**inputs):
    if "nc" not in _CACHE:
        _CACHE["nc"] = _build_program()
    nc = _CACHE["nc"]
    in_maps = _prep_inputs(inputs)
    res = run_bass_kernel_spmd(nc, in_maps, core_ids=list(range(8)))
    out = np.empty((B, T, D), np.float32)
    for c in range(8):
        b, o = c // 2, c % 2
        out[b, o * TQ:(o + 1) * TQ] = res.results[c]["out"]
    return out
